# revision 29
# baseline (speedup 1.0000x reference)
"""Trainium2 Bass kernel for nn_Attention (GQA + RoPE + softmax-n + causal).

Full inputs -> shard DP2(batch) x TP4(heads) across 8 cores -> gather+sum.

Per-core device program (matmuls bf16 by default, PSUM fp32):
  Fully software-pipelined single phase. Projection work for x^T chunk
  sc+1 and the output projection for q-chunk qc-1 are emitted as filler
  thunks interleaved between attention tiles of q-chunk qc, so the PE
  always has ready work while Act(exp)/DVE latencies drain.

  proj(sc): Q^T/K^T = w.T @ x^T, RoPE via sign-folded tables + DMA
            partition half-swap; V directly in natural [s, hd] layout
            (x^T tile as lhsT).
  att(qc), per head: scores^T[k,q] = K^T.T @ Q^T (causal banded),
            E = exp(scores^T) on Act, diag masked by triangle multiply,
            esum += E on DVE, out^T[hd,q] += V.T @ E in PSUM,
            denom = ones.T @ esum (one matmul) + 1.0 (softmax-n phantom),
            out^T *= broadcast(1/denom).
  wo(qc):   out[s,:] += oc_h.T @ wo_h, staged to [128, D] tiles, one
            DMA per seq tile.

Host: out[b] = sum over 4 TP shards of out_partial.
"""
import sys
import numpy as np

sys.path.insert(0, "/opt/trn_rl_repo")

import concourse.bass as bass
import concourse.bacc as bacc
import concourse.mybir as mybir
import concourse.tile as tile
from concourse import bass_utils
from concourse._compat import with_exitstack

F32 = mybir.dt.float32
F32R = mybir.dt.float32r
BF16 = mybir.dt.bfloat16
EXP = mybir.ActivationFunctionType.Exp

B, S, D = 2, 2048, 2048
N_HEADS, N_KV_HEADS, HD = 16, 8, 128
TP = 4                      # tensor-parallel ways (x DP2 over batch = 8 cores)
QF = 4 * HD                 # per-core q feature cols   (512)
KF = 2 * HD                 # per-core k/v feature cols (256)
NQT = S // 128              # 16 seq tiles
NQC = S // 512              # 4  q-chunks
ND = D // 128               # 16 contraction tiles
NSC = S // 512              # 4  x^T stream chunks

_CACHE = {}

# matmul dtype config per stage: "f32r" or "bf16"
import os
CFG = {"proj": os.environ.get("K_PROJ", "bf16"),
       "attn": os.environ.get("K_ATTN", "bf16"),  # qkT / scores inputs
       "e": os.environ.get("K_E", "bf16"),        # e / vnat / esum / AV
       "wo": os.environ.get("K_WO", "bf16")}


def _dt(stage):
    return F32R if CFG[stage] == "f32r" else BF16


def _npdt(stage):
    import ml_dtypes
    return np.float32 if CFG[stage] == "f32r" else ml_dtypes.bfloat16


def _build(bench_reps=None):
    nc = bacc.Bacc("TRN2", target_bir_lowering=False, debug=False)

    PJ, AT, ED, WD = _dt("proj"), _dt("attn"), _dt("e"), _dt("wo")
    names = [("xT", [D, S], PJ), ("wq", [D, QF], PJ), ("wk", [D, KF], PJ),
             ("wv", [D, KF], PJ), ("wo", [QF, D], WD),
             ("c2", [128, S], F32), ("g", [128, S], F32),
             ("tri", [128, 128], ED), ("tri2", [128, 256], ED),
             ("ones128", [128, 1], ED)]
    kind = "Internal" if bench_reps else "ExternalInput"
    io = {n: nc.dram_tensor(n, sh, dt, kind=kind) for n, sh, dt in names}
    if bench_reps:
        io["dummy"] = nc.dram_tensor("bench_in", [128, 1], F32,
                                     kind="ExternalInput")
    io["out"] = nc.dram_tensor("out", [S, D], F32, kind="ExternalOutput")

    with tile.TileContext(nc) as tc:
        if bench_reps:
            # fill internal DRAM inputs with benign constants (avoid
            # garbage -> denormal/NaN timing artifacts)
            with tc.tile_pool(name="fillp", bufs=1) as fp:
                f3t = fp.tile([128, 2048], F32, tag="fill32")
                fbt = fp.tile([128, 2048], BF16, tag="fillb")
                nc.gpsimd.memset(f3t[:], 0.001)
                nc.gpsimd.memset(fbt[:], 0.001)
                for n, sh, dt in names:
                    r, c = sh
                    for r0 in range(0, r, 128):
                        rr = min(128, r - r0)
                        for c0 in range(0, c, 2048):
                            cc = min(2048, c - c0)
                            if dt == BF16:
                                srcap = fbt[:rr, :cc]
                            elif dt == F32R:
                                srcap = f3t[:rr, :cc].bitcast(F32R)
                            else:
                                srcap = f3t[:rr, :cc]
                            nc.sync.dma_start(io[n][r0:r0 + rr, c0:c0 + cc],
                                              srcap)
        with tc.tile_pool(name="persist", bufs=1) as persist:
            env = _setup(persist, nc, io)
            if bench_reps and bench_reps > 1:
                with tc.For_i(0, bench_reps, 1):
                    _emit(tc, nc, io, env)
            else:
                _emit(tc, nc, io, env)
    nc.compile()
    return nc


def _setup(persist, nc, io):
    """Persistent tiles + initial const/weight loads (outside the bench
    loop). Returns the tile environment used by _emit."""
    PJ, AT, ED, WD = _dt("proj"), _dt("attn"), _dt("e"), _dt("wo")

    env = {}
    env["tri"] = persist.tile([128, 128], ED, tag="tri", name="tri")
    env["tri2"] = persist.tile([128, 256], ED, tag="tri2", name="tri2")
    env["ones128"] = persist.tile([128, 1], ED, tag="ones128",
                                  name="ones128")
    env["c2sb"] = persist.tile([128, S], F32, tag="c2sb", name="c2sb")
    env["gsb"] = persist.tile([128, S], F32, tag="gsb", name="gsb")
    # weights, whole-tensor DMA each: [128, t, cols] with t = row-tile index
    env["wqb"] = persist.tile([128, ND, QF], PJ, tag="wqb", name="wqb")
    env["wkb"] = persist.tile([128, ND, KF], PJ, tag="wkb", name="wkb")
    env["wvb"] = persist.tile([128, ND, KF], PJ, tag="wvb", name="wvb")
    env["wob"] = persist.tile([128, 4, D], WD, tag="wob", name="wob")
    _load_consts_weights(nc, io, env)

    # rotated Q^T/K^T: 6 head tiles [128, S]; V natural: 16 tiles [128, KF]
    env["qkT"] = [persist.tile([128, S], AT, tag=f"qkT{f}", name=f"qkT{f}")
                  for f in range(6)]
    env["vnat"] = [persist.tile([128, KF], ED, tag=f"vnat{st}",
                                name=f"vnat{st}") for st in range(NQT)]
    return env


def _load_consts_weights(nc, io, env, part=None):
    """part=None: everything. part="early": tensors whose last in-iteration
    read ends with proj(3) — safe to reload while tail attention runs.
    part="late": tensors read until the end of the body (reload last)."""
    if part in (None, "early"):
        for name, t in [("c2", env["c2sb"]), ("g", env["gsb"])]:
            nc.sync.dma_start(t[:], io[name][:])
        for name, t in [("wq", env["wqb"]), ("wk", env["wkb"]),
                        ("wv", env["wvb"])]:
            nc.sync.dma_start(t[:],
                              io[name].rearrange("(t p) q -> p t q", p=128))
    if part in (None, "late"):
        for name, t in [("tri", env["tri"]), ("tri2", env["tri2"]),
                        ("ones128", env["ones128"])]:
            nc.sync.dma_start(t[:], io[name][:])
        nc.sync.dma_start(env["wob"][:],
                          io["wo"].rearrange("(t p) q -> p t q", p=128))


@with_exitstack
def _emit(ctx, tc, nc, io, env):
    ts = bass.ts
    PJ, AT, ED, WD = _dt("proj"), _dt("attn"), _dt("e"), _dt("wo")
    tri, tri2, ones128 = env["tri"], env["tri2"], env["ones128"]
    c2sb, gsb = env["c2sb"], env["gsb"]
    wqb, wkb, wvb, wob = env["wqb"], env["wkb"], env["wvb"], env["wob"]
    qkT, vnat = env["qkT"], env["vnat"]

    peonly = bool(os.environ.get("K_PEONLY"))
    xtp = ctx.enter_context(tc.tile_pool(name="xtp", bufs=2))
    ropep = ctx.enter_context(tc.tile_pool(name="rope", bufs=3))
    ep = ctx.enter_context(tc.tile_pool(name="ep", bufs=8))
    esump = ctx.enter_context(tc.tile_pool(name="esump", bufs=3))
    ocp = ctx.enter_context(tc.tile_pool(name="ocp", bufs=8))
    finp = ctx.enter_context(tc.tile_pool(name="fin", bufs=2))
    stagep = ctx.enter_context(tc.tile_pool(name="stagep", bufs=2))
    psp = ctx.enter_context(tc.tile_pool(name="psp", bufs=1, space="PSUM"))

    xt_tiles = {}

    def load_xt(sc):
        xtb = xtp.tile([128, ND, 512], PJ, tag="xt", name=f"xt{sc}", bufs=2)
        cs = ts(sc, 512)
        nc.gpsimd.dma_start(
            xtb[:], io["xT"][:, cs].rearrange("(t p) s -> p t s", p=128))
        xt_tiles[sc] = xtb

    def rope(f, ps, cs):
        a = ropep.tile([128, 512], F32, tag="ropeA")
        b = ropep.tile([128, 512], F32, tag="ropeB")
        bsw = ropep.tile([128, 512], F32, tag="ropeBsw")
        nc.vector.tensor_mul(a[:], ps[:], c2sb[:, cs])
        nc.vector.tensor_mul(b[:], ps[:], gsb[:, cs])
        nc.gpsimd.dma_start(bsw[0:64, :], b[64:128, :])
        nc.gpsimd.dma_start(bsw[64:128, :], b[0:64, :])
        nc.vector.tensor_add(qkT[f][:, cs], a[:], bsw[:])

    def proj_thunks(sc, ps_tag="proj", ps_bufs=1, alt_tag=None):
        """Filler thunks computing projection chunk sc; first thunk issues
        the x^T DMA for chunk sc+1. With alt_tag, chains alternate between
        two single-buf tags for 2-deep pipelining (only safe when the
        alt tag has no other concurrent users)."""
        th = []
        cs = ts(sc, 512)
        if sc + 1 < NSC:
            th.append(lambda s=sc: load_xt(s + 1))

        def chain_tag(ci):
            if alt_tag is not None and ci % 2 == 1:
                return alt_tag
            return ps_tag
        for f in range(6):
            cell = {}

            def seg(lo, hi, f=f, cell=cell):
                xtb = xt_tiles[sc]
                wt, fo = (wqb, f * 128) if f < 4 else (wkb, (f - 4) * 128)
                if lo == 0:
                    cell["ps"] = psp.tile([128, 512], F32, tag=chain_tag(f),
                                          name=f"ps{sc}_{f}", bufs=ps_bufs)
                ps = cell["ps"]
                for d in range(lo, hi):
                    nc.tensor.matmul(ps[:], wt[:, d, fo:fo + 128],
                                     xtb[:, d, :],
                                     start=(d == 0), stop=(d == ND - 1))
                if hi == ND:
                    rope(f, ps, cs)
            for lo in range(0, ND, 4):
                th.append(lambda lo=lo, hi=lo + 4, seg=seg: seg(lo, hi))
        for sub in range(4):
            st = sc * 4 + sub
            cell = {}

            def vseg(lo, hi, sub=sub, st=st, cell=cell):
                xtb = xt_tiles[sc]
                if lo == 0:
                    cell["ps"] = psp.tile([128, KF], F32,
                                          tag=chain_tag(6 + sub),
                                          name=f"vps{st}", bufs=ps_bufs)
                ps = cell["ps"]
                for d in range(lo, hi):
                    nc.tensor.matmul(ps[:], xtb[:, d, ts(sub, 128)],
                                     wvb[:, d, :],
                                     start=(d == 0), stop=(d == ND - 1))
                if hi == ND:
                    nc.vector.tensor_copy(vnat[st][:], ps[:])
            for lo in range(0, ND, 8):
                th.append(lambda lo=lo, hi=lo + 8, vseg=vseg: vseg(lo, hi))
        return th

    def wo_thunks(qc, oc, copy_eng, ps_tag="wo3", ps_bufs=1):
        """Filler thunks for q-chunk qc's output projection."""
        th = []
        for sub in range(4):
            st = qc * 4 + sub
            cell = {}

            def chain(dc, sub=sub, st=st, cell=cell, oc=oc):
                if dc == 0:
                    cell["stage"] = stagep.tile([128, D], F32, tag="stage",
                                                name=f"stage{st}", bufs=2)
                ps3 = psp.tile([128, 512], F32, tag=ps_tag, bufs=ps_bufs,
                               name="ps3")
                for hf in range(4):
                    nc.tensor.matmul(ps3[:], oc[hf][:, ts(sub, 128)],
                                     wob[:, hf, ts(dc, 512)],
                                     start=(hf == 0), stop=(hf == 3))
                if copy_eng == "vector":
                    nc.vector.tensor_copy(cell["stage"][:, ts(dc, 512)],
                                          ps3[:])
                else:
                    nc.scalar.copy(cell["stage"][:, ts(dc, 512)], ps3[:])
                if dc == 3:
                    nc.sync.dma_start(io["out"][ts(st, 128), :],
                                      cell["stage"][:])
            for dc in range(4):
                th.append(lambda dc=dc, chain=chain: chain(dc))
        return th

    def emit_att(qc, fillers):
        """Attention for q-chunk qc with filler thunks drained evenly
        between tiles."""
        qs = qc * 512
        nkt = 4 * (qc + 1)
        total_tiles = 4 * nkt
        tile_i = 0
        drained = 0
        oc = [None] * 4
        out_pss = [None] * 4
        esums = [None] * 4
        pend = [None]

        def den_and_norm(j):
            if peonly:
                o = ocp.tile([128, 512], WD, tag="oc", name=f"oc{j}_{qc}")
                nc.vector.tensor_copy(o[:], out_pss[j][:])
                oc[j] = o
                return
            den_ps = psp.tile([1, 512], F32, tag="sc2", bufs=2,
                              name="den_ps")
            nc.tensor.matmul(den_ps[:], ones128[:], esums[j][:],
                             start=True, stop=True)
            denf = finp.tile([1, 512], F32, tag="denf")
            nc.vector.tensor_scalar_add(denf[:], den_ps[:], 1.0)
            rec = finp.tile([1, 512], F32, tag="rec")
            with nc.allow_low_precision(reason="recip of denom"):
                nc.vector.reciprocal(rec[:], denf[:])
            bcs = finp.tile([128, 512], F32, tag="bcs")
            nc.gpsimd.partition_broadcast(bcs[:], rec[:])
            o = ocp.tile([128, 512], WD, tag="oc", name=f"oc{j}_{qc}")
            nc.vector.tensor_mul(o[:], out_pss[j][:], bcs[:])
            oc[j] = o

        for h in range(4):
            gkv = h // 2
            qT, kT = qkT[h], qkT[4 + gkv]
            out_ps = psp.tile([128, 512], F32, tag="out", bufs=2)
            out_pss[h] = out_ps
            esum = esump.tile([128, 512], ED, tag="esum")
            esums[h] = esum
            # process k-tiles in pairs: two scores matmuls into one
            # 2-bank PSUM tile, ONE exp instruction for both
            for kp in range(nkt // 2):
                kt0, kt1 = 2 * kp, 2 * kp + 1
                m0 = max(0, 128 * kt0 - qs)
                m1 = max(0, 128 * kt1 - qs)
                # scores matmul region: fp32r needs N>=256 for full rate
                s0 = 256 if (m0 == 384 and AT == F32R) else m0
                s1 = 256 if (m1 == 384 and AT == F32R) else m1
                e0 = s0 if ED == F32R else m0
                e1 = s1 if ED == F32R else m1
                sc2 = psp.tile([128, 1024], F32, tag="sc2", bufs=2,
                               name="sc2")
                nc.tensor.matmul(sc2[:, s0:512], kT[:, ts(kt0, 128)],
                                 qT[:, qs + s0:qs + 512],
                                 start=True, stop=True)
                nc.tensor.matmul(sc2[:, 512 + s1:1024], kT[:, ts(kt1, 128)],
                                 qT[:, qs + s1:qs + 512],
                                 start=True, stop=True)
                e2 = ep.tile([128, 1024], ED, tag="e")
                if peonly:
                    if s1 > 0:
                        nc.scalar.copy(e2[:, s0:512], sc2[:, s0:512])
                        nc.scalar.copy(e2[:, 512 + s1:], sc2[:, 512 + s1:])
                    else:
                        nc.scalar.copy(e2[:, s0:], sc2[:, s0:])
                elif s1 > 0:
                    # diag pair: skip the uninitialized PSUM gap between
                    # the two halves
                    nc.scalar.activation(e2[:, s0:512], sc2[:, s0:512], EXP)
                    nc.scalar.activation(e2[:, 512 + s1:], sc2[:, 512 + s1:],
                                         EXP)
                else:
                    nc.scalar.activation(e2[:, s0:], sc2[:, s0:], EXP)
                if not peonly:
                    if kt0 >= 4 * qc:          # diag masks
                        nc.vector.tensor_mul(e2[:, m0:m0 + 128],
                                             e2[:, m0:m0 + 128], tri[:])
                    if kt1 >= 4 * qc:
                        if m1 == 384 and e1 == 256:
                            nc.vector.tensor_mul(e2[:, 768:1024],
                                                 e2[:, 768:1024], tri2[:])
                        else:
                            nc.vector.tensor_mul(
                                e2[:, 512 + m1:512 + m1 + 128],
                                e2[:, 512 + m1:512 + m1 + 128], tri[:])
                nc.tensor.matmul(out_ps[:, e0:],
                                 vnat[kt0][:, gkv * 128:(gkv + 1) * 128],
                                 e2[:, e0:512],
                                 start=(kp == 0), stop=False)
                nc.tensor.matmul(out_ps[:, e1:],
                                 vnat[kt1][:, gkv * 128:(gkv + 1) * 128],
                                 e2[:, 512 + e1:1024],
                                 start=False, stop=(kp == nkt // 2 - 1))
                if not peonly:
                    # defer this pair's esum accumulation by one pair so
                    # the next pair's diag masks (AV-critical) aren't
                    # queued behind it on DVE
                    def adds(kp=kp, e2=e2, esum=esum, e0=e0, e1=e1):
                        if kp == 0:
                            nc.vector.tensor_copy(esum[:], e2[:, 0:512])
                        else:
                            nc.vector.tensor_add(esum[:, e0:], esum[:, e0:],
                                                 e2[:, e0:512])
                        nc.vector.tensor_add(esum[:, e1:], esum[:, e1:],
                                             e2[:, 512 + e1:1024])
                    if pend[0] is not None:
                        pend[0]()
                    pend[0] = adds
                if kp == 0 and h > 0:
                    den_and_norm(h - 1)
                tile_i += 2
                want = (len(fillers) * tile_i) // total_tiles
                while drained < want:
                    fillers[drained]()
                    drained += 1
        if pend[0] is not None:
            pend[0]()
        den_and_norm(3)
        while drained < len(fillers):
            fillers[drained]()
            drained += 1
        return oc

    def merge(a, b):
        """Round-robin merge of two thunk lists, proportional."""
        if not b:
            return list(a)
        out, ia, ib = [], 0, 0
        n = len(a) + len(b)
        for i in range(n):
            if ia * len(b) <= ib * len(a) and ia < len(a):
                out.append(a[ia]); ia += 1
            elif ib < len(b):
                out.append(b[ib]); ib += 1
            else:
                out.append(a[ia]); ia += 1
        return out

    # ---------------- emission schedule ----------------
    load_xt(0)
    # head projection runs back-to-back; "out" psum tag is free until
    # att(0), giving it 2 banks of pipelining
    for t in proj_thunks(0, ps_tag="out", ps_bufs=2):
        t()                       # first thunk also issues load_xt(1)
    oc_prev = None
    for qc in range(NQC):
        pj = proj_thunks(qc + 1) if qc + 1 < NSC else []
        if qc == NQC - 1:
            # reload x-chunk-side consts/weights for the next bench-loop
            # iteration while the tail attention runs (their last reads
            # are behind us; WAR-ordered by the tile framework)
            pj = [lambda: _load_consts_weights(nc, io, env, "early")]
        eng = "vector" if qc >= 2 else "scalar"
        wt = wo_thunks(qc - 1, oc_prev, eng) if oc_prev is not None else []
        oc_prev = emit_att(qc, merge(pj, wt))
    # tail output projection: "proj"/"sc2" tags are idle now; use the
    # "out" tag (2 bufs) so back-to-back chains don't stall on drains
    for t in wo_thunks(NQC - 1, oc_prev, "vector", ps_tag="out", ps_bufs=2):
        t()
    # tensors read until the very end reload after their last use
    _load_consts_weights(nc, io, env, "late")


def _host_prep(x, freqs_cos, freqs_sin, wq, wk, wv, wo):
    """Build the 8 per-core input maps."""
    # de-interleave perm within every 128-col head block: [0,2,..,126,1,3,..,127]
    p128 = np.concatenate([np.arange(0, 128, 2), np.arange(1, 128, 2)])
    permq = np.concatenate([hb * 128 + p128 for hb in range(N_HEADS)])
    permk = np.concatenate([hb * 128 + p128 for hb in range(N_KV_HEADS)])
    wq_p = (wq / np.sqrt(np.float32(HD)))[:, permq]
    wk_p = wk[:, permk]

    cosT = np.ascontiguousarray(freqs_cos.T)            # [64, S]
    sinT = np.ascontiguousarray(freqs_sin.T)
    c2 = np.concatenate([cosT, cosT], 0).astype(np.float32)   # [128, S]
    gtab = np.concatenate([sinT, -sinT], 0).astype(np.float32)

    ii, jj = np.meshgrid(np.arange(128), np.arange(128), indexing="ij")
    tri = (ii <= jj).astype(np.float32)                 # [k, q] allow k<=q

    tri2 = np.concatenate([np.zeros((128, 128), np.float32), tri], 1)
    ed, pj, wd = _npdt("e"), _npdt("proj"), _npdt("wo")
    common = {
        "c2": c2, "g": gtab, "tri": tri.astype(ed), "tri2": tri2.astype(ed),
        "ones128": np.ones((128, 1), ed),
    }
    in_maps = []
    for core in range(8):
        b, t = divmod(core, TP)
        in_maps.append({
            "xT": np.ascontiguousarray(x[b].T).astype(pj),
            "wq": np.ascontiguousarray(wq_p[:, t * QF:(t + 1) * QF]).astype(pj),
            "wk": np.ascontiguousarray(wk_p[:, t * KF:(t + 1) * KF]).astype(pj),
            "wv": np.ascontiguousarray(wv[:, t * KF:(t + 1) * KF]).astype(pj),
            "wo": np.ascontiguousarray(wo[t * QF:(t + 1) * QF, :]).astype(wd),
            **common,
        })
    return in_maps


def kernel(x, freqs_cos, freqs_sin, wq, wk, wv, wo, _trace=False):
    in_maps = _host_prep(np.asarray(x, np.float32),
                         np.asarray(freqs_cos, np.float32),
                         np.asarray(freqs_sin, np.float32),
                         np.asarray(wq, np.float32), np.asarray(wk, np.float32),
                         np.asarray(wv, np.float32), np.asarray(wo, np.float32))
    if "nc" not in _CACHE:
        _CACHE["nc"] = _build()
    res = bass_utils.run_bass_kernel_spmd(_CACHE["nc"], in_maps, list(range(8)),
                                          trace=_trace)
    _CACHE["last_result"] = res
    out = np.zeros((B, S, D), np.float32)
    for core in range(8):
        b = core // TP
        out[b] += res.results[core]["out"]
    return out
